# revision 2
# baseline (speedup 1.0000x reference)
"""Trainium2 kernel for nn_AlephPracticalEngine (sparse_attention).

The reference computes out = fhh(fhh(x) * w + gather-einsum) / DIM, which is
linear in x: out = x @ W^T with W = M (diag(w) + S) M / DIM, where M is the
fhh butterfly matrix and S is scatter-added from the engram tables. The
(x-independent) weight fold runs on host; the device does one dense
[2048,4096] x [4096,4096] bf16 matmul, sharded 4 (batch) x 2 (features)
across 8 NeuronCores.

Per core: out_c[512,2048] = x_c[512,4096] @ W_c[2048,4096]^T via a custom
tile kernel: resident SBUF operands, K-contiguous loop, one stationary
weight load per 4 matmuls (4 PSUM banks), HAM-warmup matmuls in the head.
"""

import numpy as np
import ml_dtypes

DIM = 4096
BATCH = 2048
N_CORES = 8
ROW_GROUPS = 4  # batch groups
COL_GROUPS = 2  # output-feature groups
M_C = BATCH // ROW_GROUPS  # 512 batch rows per core
N_C = DIM // COL_GROUPS  # 2048 output features per core

P = 128
NFREE = 512
KT = DIM // P  # 32 k subtiles
MT = M_C // P  # 4 m chunks
NT = N_C // NFREE  # 4 n banks
N_WARMUP = 10

_compiled_nc = None


def _fhh_np(x):
    """numpy mirror of the reference fhh butterfly (last axis)."""
    n = x.shape[-1]
    if n == 1:
        return x
    half = n // 2
    left, right = x[..., :half], x[..., half:]
    h_sum = left + right
    h_diff = left - right
    h_diff2 = h_diff + np.roll(h_diff, shift=1, axis=-1)
    return np.concatenate([_fhh_np(h_sum), _fhh_np(h_diff2)], axis=-1)


def _build_Wt(spectral_weights, engram_values, engram_indices, dtype=np.float32):
    """W^T [k, d] for out = x @ W^T, W = M (diag(w) + S) M / DIM."""
    w = np.asarray(spectral_weights, dtype)
    val = np.asarray(engram_values, dtype)
    idx = np.asarray(engram_indices).astype(np.int64)
    D, K = val.shape
    # M^T = fhh(I):  fhh(e_j)[i] = M[i, j]
    M = np.ascontiguousarray(_fhh_np(np.eye(D, dtype=dtype)).T)
    AM = w[:, None] * M
    for k in range(K):
        AM += val[:, k][:, None] * M[idx[:, k], :]
    # W = fhh applied along axis 0 of AM, / D;  W^T = fhh(AM^T) / D
    Wt = _fhh_np(np.ascontiguousarray(AM.T)) / D
    return np.ascontiguousarray(Wt)


def _build_device_kernel(tc, xt_ap, wt_ap, out_ap):
    import concourse.mybir as mybir

    nc = tc.nc
    bf16 = mybir.dt.bfloat16
    f32 = mybir.dt.float32

    with (
        tc.tile_pool(name="const", bufs=1) as const,
        tc.tile_pool(name="xpool", bufs=1) as xpool,
        tc.tile_pool(name="wpool", bufs=1) as wpool,
        tc.tile_pool(name="stage", bufs=6) as stage,
        tc.tile_pool(name="psum", bufs=2, space="PSUM") as psum,
    ):
        # HAM warmup: dummy matmuls with no DMA dependency fill the head wait.
        wz = const.tile([P, NFREE], bf16, tag="wz")
        nc.any.memset(wz[:], 0.0)
        warm_ps = psum.tile([P, NFREE], f32, tag="acc")
        for _ in range(N_WARMUP):
            nc.tensor.matmul(warm_ps[:], wz[:, :P], wz[:], start=True, stop=True)

        # Resident loads; k-interleaved so the first matmul's deps land first.
        xt_sb = []
        wt_sb = []
        for k in range(KT):
            xk = xpool.tile([P, MT * P], bf16, tag=f"x{k}")
            nc.sync.dma_start(xk[:], xt_ap[k])
            xt_sb.append(xk)
            row = []
            for n in range(NT):
                wkn = wpool.tile([P, NFREE], bf16, tag=f"w{k}_{n}")
                nc.sync.dma_start(wkn[:], wt_ap[k * NT + n])
                row.append(wkn)
            wt_sb.append(row)

        for m in range(MT):
            acc = psum.tile([P, NT * NFREE], f32, tag="acc")
            for k in range(KT):
                lhsT = xt_sb[k][:, m * P : (m + 1) * P]
                for n in range(NT):
                    nc.tensor.matmul(
                        acc[:, n * NFREE : (n + 1) * NFREE],
                        lhsT,
                        wt_sb[k][n][:],
                        start=(k == 0),
                        stop=(k == KT - 1),
                    )
            for n in range(NT):
                st = stage.tile([P, NFREE], f32, tag="st")
                nc.vector.tensor_copy(st[:], acc[:, n * NFREE : (n + 1) * NFREE])
                nc.sync.dma_start(out_ap[m, n], st[:])


def _get_compiled():
    global _compiled_nc
    if _compiled_nc is None:
        import concourse.mybir as mybir
        import concourse.tile as tile
        from concourse import bacc

        nc = bacc.Bacc(
            "TRN2",
            target_bir_lowering=False,
            debug=False,
            num_devices=N_CORES,
        )
        xt = nc.dram_tensor(
            "xt", [KT, P, MT * P], mybir.dt.bfloat16, kind="ExternalInput"
        )
        wt = nc.dram_tensor(
            "wt", [KT * NT, P, NFREE], mybir.dt.bfloat16, kind="ExternalInput"
        )
        out = nc.dram_tensor(
            "out", [MT, NT, P, NFREE], mybir.dt.float32, kind="ExternalOutput"
        )
        with tile.TileContext(nc) as tc:
            _build_device_kernel(tc, xt.ap(), wt.ap(), out.ap())
        nc.compile()
        _compiled_nc = nc
    return _compiled_nc


def _prepare_in_maps(inputs):
    x = np.asarray(inputs["x"], np.float32)
    Wt = _build_Wt(
        inputs["spectral_weights"], inputs["engram_values"], inputs["engram_indices"]
    )  # [DIM(k), DIM(d)] f32
    xb = x.astype(ml_dtypes.bfloat16)
    Wtb = Wt.astype(ml_dtypes.bfloat16)
    in_maps = []
    for c in range(N_CORES):
        r, f = divmod(c, COL_GROUPS)
        # xt pieces: x_c.T reshaped [KT, P, M_C]
        xt = np.ascontiguousarray(xb[r * M_C : (r + 1) * M_C, :].T).reshape(
            KT, P, M_C
        )
        # wt pieces: W_c.T [DIM, N_C] -> [KT, P, NT, NFREE] -> [KT*NT, P, NFREE]
        wslab = np.ascontiguousarray(Wtb[:, f * N_C : (f + 1) * N_C])
        wt = np.ascontiguousarray(
            wslab.reshape(KT, P, NT, NFREE).transpose(0, 2, 1, 3)
        ).reshape(KT * NT, P, NFREE)
        in_maps.append({"xt": xt, "wt": wt})
    return in_maps


def _run(in_maps, trace=False):
    from concourse.bass_utils import run_bass_kernel_spmd

    nc = _get_compiled()
    return run_bass_kernel_spmd(
        nc, in_maps, core_ids=list(range(N_CORES)), trace=trace
    )


def _assemble(results):
    out = np.empty((BATCH, DIM), np.float32)
    for c in range(N_CORES):
        r, f = divmod(c, COL_GROUPS)
        piece = results[c]["out"]  # [MT, NT, P, NFREE]
        core_out = piece.transpose(0, 2, 1, 3).reshape(M_C, N_C)
        out[r * M_C : (r + 1) * M_C, f * N_C : (f + 1) * N_C] = core_out
    return out


def kernel(**inputs):
    res = _run(_prepare_in_maps(inputs))
    return _assemble(res.results)


# revision 4
# speedup vs baseline: 1.1587x; 1.1587x over previous
"""Trainium2 kernel for nn_AlephPracticalEngine (sparse_attention).

The reference computes out = fhh(fhh(x) * w + gather-einsum) / DIM, which is
linear in x: out = x @ W^T with W = M (diag(w) + S) M / DIM, where M is the
fhh butterfly matrix and S is scatter-added from the engram tables. The
(x-independent) weight fold runs on host; the device does one dense
[2048,4096] x [4096,4096] bf16 matmul, sharded 4 (batch) x 2 (features)
across 8 NeuronCores.

Per core: out_c[512,2048] = x_c[512,4096] @ W_c[2048,4096]^T. Custom tile
kernel: two n-half passes (PSUM = 4 m-tiles x 2 banks per pass) so weight
DMA spreads across the whole run; one stationary load per 2 matmuls;
512KB DMA pieces; HAM-warmup matmuls kept live via a scratch output.
"""

import numpy as np
import ml_dtypes

DIM = 4096
BATCH = 2048
N_CORES = 8
ROW_GROUPS = 4  # batch groups
COL_GROUPS = 2  # output-feature groups
M_C = BATCH // ROW_GROUPS  # 512 batch rows per core
N_C = DIM // COL_GROUPS  # 2048 output features per core

P = 128
NFREE = 512
KT = DIM // P  # 32 k subtiles
MT = M_C // P  # 4 m chunks
NT = N_C // NFREE  # 4 n banks
N_PASS = 2  # n-half passes (2 banks each)
N_WARMUP = 8

_compiled_nc = None


def _fhh_np(x):
    """numpy mirror of the reference fhh butterfly (last axis)."""
    n = x.shape[-1]
    if n == 1:
        return x
    half = n // 2
    left, right = x[..., :half], x[..., half:]
    h_sum = left + right
    h_diff = left - right
    h_diff2 = h_diff + np.roll(h_diff, shift=1, axis=-1)
    return np.concatenate([_fhh_np(h_sum), _fhh_np(h_diff2)], axis=-1)


def _build_Wt(spectral_weights, engram_values, engram_indices, dtype=np.float32):
    """W^T [k, d] for out = x @ W^T, W = M (diag(w) + S) M / DIM."""
    w = np.asarray(spectral_weights, dtype)
    val = np.asarray(engram_values, dtype)
    idx = np.asarray(engram_indices).astype(np.int64)
    D, K = val.shape
    # M^T = fhh(I):  fhh(e_j)[i] = M[i, j]
    M = np.ascontiguousarray(_fhh_np(np.eye(D, dtype=dtype)).T)
    AM = w[:, None] * M
    for k in range(K):
        AM += val[:, k][:, None] * M[idx[:, k], :]
    # W = fhh applied along axis 0 of AM, / D;  W^T = fhh(AM^T) / D
    Wt = _fhh_np(np.ascontiguousarray(AM.T)) / D
    return np.ascontiguousarray(Wt)


def _build_device_kernel(tc, xt_ap, wt_ap, out_ap, warm_ap):
    import concourse.mybir as mybir

    nc = tc.nc
    bf16 = mybir.dt.bfloat16
    f32 = mybir.dt.float32

    with (
        tc.tile_pool(name="const", bufs=1) as const,
        tc.tile_pool(name="xpool", bufs=1) as xpool,
        tc.tile_pool(name="wpool", bufs=1) as wpool,
        tc.tile_pool(name="stage", bufs=6) as stage,
        tc.tile_pool(name="psum", bufs=4, space="PSUM") as psum,
    ):
        # HAM warmup: one live accumulation group with no DMA dependency,
        # kept alive (vs DCE) by a tiny copy-out to a scratch output.
        wz = const.tile([P, NFREE], bf16, tag="wz")
        nc.any.memset(wz[:], 0.0)
        warm_ps = psum.tile([P, NFREE], f32, tag="acc")
        for i in range(N_WARMUP):
            nc.tensor.matmul(
                warm_ps[:],
                wz[:, :P],
                wz[:],
                start=(i == 0),
                stop=(i == N_WARMUP - 1),
            )
        warm_st = stage.tile([P, 8], f32, tag="warm_st")
        nc.vector.tensor_copy(warm_st[:], warm_ps[:, :8])
        nc.sync.dma_start(warm_ap, warm_st[:])

        # Resident loads, 512KB pieces, emitted in consumption order:
        # pass A (xt + first wt half) interleaved, then pass B's wt half.
        xt_sb = []  # 8 tiles [P, 4*NFREE]; k-slab k at cols (k%4)*NFREE
        wt_sb = [[], []]  # per pass: 16 tiles [P, 2*1024]; k at (k%2)*1024
        for kk in range(KT // 4):
            xk = xpool.tile([P, 4 * NFREE], bf16, tag=f"x{kk}")
            nc.sync.dma_start(xk[:], xt_ap[kk])
            xt_sb.append(xk)
            for h in range(2):
                wkn = wpool.tile([P, 2 * NT // N_PASS * NFREE], bf16, tag=f"wa{kk}_{h}")
                nc.sync.dma_start(wkn[:], wt_ap[0, 2 * kk + h])
                wt_sb[0].append(wkn)
        for kk in range(KT // 2):
            wkn = wpool.tile([P, 2 * NT // N_PASS * NFREE], bf16, tag=f"wb{kk}")
            nc.sync.dma_start(wkn[:], wt_ap[1, kk])
            wt_sb[1].append(wkn)

        nhalf = NT // N_PASS  # 2 banks per pass
        for p in range(N_PASS):
            accs = [
                psum.tile([P, nhalf * NFREE], f32, tag="acc", name=f"acc{p}_{m}")
                for m in range(MT)
            ]
            for k in range(KT):
                wt_tile = wt_sb[p][k // 2]
                for m in range(MT):
                    lhsT = xt_sb[k // 4][:, (k % 4) * NFREE + m * P :][:, :P]
                    for j in range(nhalf):
                        nc.tensor.matmul(
                            accs[m][:, j * NFREE : (j + 1) * NFREE],
                            lhsT,
                            wt_tile[:, (k % 2) * nhalf * NFREE + j * NFREE :][
                                :, :NFREE
                            ],
                            start=(k == 0),
                            stop=(k == KT - 1),
                        )
            for m in range(MT):
                for j in range(nhalf):
                    st = stage.tile([P, NFREE], f32, tag="st")
                    nc.vector.tensor_copy(st[:], accs[m][:, j * NFREE : (j + 1) * NFREE])
                    nc.sync.dma_start(out_ap[m, p * nhalf + j], st[:])


def _get_compiled():
    global _compiled_nc
    if _compiled_nc is None:
        import concourse.mybir as mybir
        import concourse.tile as tile
        from concourse import bacc

        nc = bacc.Bacc(
            "TRN2",
            target_bir_lowering=False,
            debug=False,
            num_devices=N_CORES,
        )
        xt = nc.dram_tensor(
            "xt", [KT // 4, P, 4 * NFREE], mybir.dt.bfloat16, kind="ExternalInput"
        )
        wt = nc.dram_tensor(
            "wt",
            [N_PASS, KT // 2, P, 2 * (NT // N_PASS) * NFREE],
            mybir.dt.bfloat16,
            kind="ExternalInput",
        )
        out = nc.dram_tensor(
            "out", [MT, NT, P, NFREE], mybir.dt.float32, kind="ExternalOutput"
        )
        warm = nc.dram_tensor("warm", [P, 8], mybir.dt.float32, kind="ExternalOutput")
        with tile.TileContext(nc) as tc:
            _build_device_kernel(tc, xt.ap(), wt.ap(), out.ap(), warm.ap())
        nc.compile()
        _compiled_nc = nc
    return _compiled_nc


def _prepare_in_maps(inputs):
    x = np.asarray(inputs["x"], np.float32)
    Wt = _build_Wt(
        inputs["spectral_weights"], inputs["engram_values"], inputs["engram_indices"]
    )  # [DIM(k), DIM(d)] f32
    xb = x.astype(ml_dtypes.bfloat16)
    Wtb = Wt.astype(ml_dtypes.bfloat16)
    in_maps = []
    for c in range(N_CORES):
        r, f = divmod(c, COL_GROUPS)
        xcT = np.ascontiguousarray(xb[r * M_C : (r + 1) * M_C, :].T)  # [DIM, M_C]
        # xt piece kk [P, 4*NFREE]: k-slabs 4kk..4kk+3 side by side
        # xcT [4096, 512] -> [8, 4, 128, 512] -> [8, 128, 4, 512]
        xt = np.ascontiguousarray(
            xcT.reshape(KT // 4, 4, P, NFREE).transpose(0, 2, 1, 3)
        ).reshape(KT // 4, P, 4 * NFREE)
        wc = Wtb[:, f * N_C : (f + 1) * N_C]  # [DIM, N_C]
        nh = NT // N_PASS * NFREE  # 1024
        # wt[p, kk] [P, 2*nh]: (k-slab 2kk cols p-half | k-slab 2kk+1 cols p-half)
        wt = np.empty((N_PASS, KT // 2, P, 2 * nh), dtype=ml_dtypes.bfloat16)
        wr = wc.reshape(KT // 2, 2, P, N_PASS, nh)  # [kk, half, P, p, nh]
        for pp in range(N_PASS):
            wt[pp] = (
                wr[:, :, :, pp, :].transpose(0, 2, 1, 3).reshape(KT // 2, P, 2 * nh)
            )
        in_maps.append({"xt": xt, "wt": np.ascontiguousarray(wt)})
    return in_maps


def _run(in_maps, trace=False):
    from concourse.bass_utils import run_bass_kernel_spmd

    nc = _get_compiled()
    return run_bass_kernel_spmd(
        nc, in_maps, core_ids=list(range(N_CORES)), trace=trace
    )


def _assemble(results):
    out = np.empty((BATCH, DIM), np.float32)
    for c in range(N_CORES):
        r, f = divmod(c, COL_GROUPS)
        piece = results[c]["out"]  # [MT, NT, P, NFREE]
        core_out = piece.transpose(0, 2, 1, 3).reshape(M_C, N_C)
        out[r * M_C : (r + 1) * M_C, f * N_C : (f + 1) * N_C] = core_out
    return out


def kernel(**inputs):
    res = _run(_prepare_in_maps(inputs))
    return _assemble(res.results)


# revision 8
# speedup vs baseline: 1.1675x; 1.0076x over previous
"""Trainium2 kernel for nn_AlephPracticalEngine (sparse_attention).

The reference computes out = fhh(fhh(x) * w + gather-einsum) / DIM, which is
linear in x: out = x @ W^T with W = M (diag(w) + S) M / DIM, where M is the
fhh butterfly matrix and S is scatter-added from the engram tables. The
(x-independent) weight fold runs on host; the device does one dense
[2048,4096] x [4096,4096] bf16 matmul, sharded 4 (batch) x 2 (features)
across 8 NeuronCores.

Per core: out_c[512,2048] = x_c[512,4096] @ W_c[2048,4096]^T. Custom tile
kernel: two n-half passes (PSUM = 4 m-tiles x 2 banks per pass) so weight
DMA spreads across the whole run; one stationary load per 2 matmuls;
512KB DMA pieces; HAM-warmup matmuls kept live via a scratch output.
"""

import numpy as np
import ml_dtypes

DIM = 4096
BATCH = 2048
N_CORES = 8
ROW_GROUPS = 4  # batch groups
COL_GROUPS = 2  # output-feature groups
M_C = BATCH // ROW_GROUPS  # 512 batch rows per core
N_C = DIM // COL_GROUPS  # 2048 output features per core

P = 128
NFREE = 512
KT = DIM // P  # 32 k subtiles
MT = M_C // P  # 4 m chunks
NT = N_C // NFREE  # 4 n banks
N_PASS = 2  # n-half passes (2 banks each)
N_WARMUP = 8

_compiled_nc = None


def _fhh_np(x):
    """numpy mirror of the reference fhh butterfly (last axis)."""
    n = x.shape[-1]
    if n == 1:
        return x
    half = n // 2
    left, right = x[..., :half], x[..., half:]
    h_sum = left + right
    h_diff = left - right
    h_diff2 = h_diff + np.roll(h_diff, shift=1, axis=-1)
    return np.concatenate([_fhh_np(h_sum), _fhh_np(h_diff2)], axis=-1)


def _build_Wt(spectral_weights, engram_values, engram_indices, dtype=np.float32):
    """W^T [k, d] for out = x @ W^T, W = M (diag(w) + S) M / DIM."""
    w = np.asarray(spectral_weights, dtype)
    val = np.asarray(engram_values, dtype)
    idx = np.asarray(engram_indices).astype(np.int64)
    D, K = val.shape
    # M^T = fhh(I):  fhh(e_j)[i] = M[i, j]
    M = np.ascontiguousarray(_fhh_np(np.eye(D, dtype=dtype)).T)
    AM = w[:, None] * M
    for k in range(K):
        AM += val[:, k][:, None] * M[idx[:, k], :]
    # W = fhh applied along axis 0 of AM, / D;  W^T = fhh(AM^T) / D
    Wt = _fhh_np(np.ascontiguousarray(AM.T)) / D
    return np.ascontiguousarray(Wt)


def _build_device_kernel(tc, xt_ap, wt_ap, out_ap, warm_ap):
    import concourse.mybir as mybir

    nc = tc.nc
    bf16 = mybir.dt.bfloat16
    f32 = mybir.dt.float32

    with (
        tc.tile_pool(name="const", bufs=1) as const,
        tc.tile_pool(name="xpool", bufs=1) as xpool,
        tc.tile_pool(name="wpool", bufs=1) as wpool,
        tc.tile_pool(name="stage", bufs=6) as stage,
        tc.tile_pool(name="psum", bufs=2, space="PSUM") as psum,
    ):
        # HAM warmup: one live accumulation group with no DMA dependency,
        # kept alive (vs DCE) by a tiny copy-out to a scratch output.
        wz = const.tile([P, NFREE], bf16, tag="wz")
        nc.any.memset(wz[:], 0.0)
        warm_ps = psum.tile([P, NFREE], f32, tag="acc")
        for i in range(N_WARMUP):
            nc.tensor.matmul(
                warm_ps[:],
                wz[:, :P],
                wz[:],
                start=(i == 0),
                stop=(i == N_WARMUP - 1),
            )
        warm_st = stage.tile([P, 8], f32, tag="warm_st")
        nc.vector.tensor_copy(warm_st[:], warm_ps[:, :8])
        nc.sync.dma_start(warm_ap, warm_st[:])

        # Resident loads in consumption order. xt split by m-half so pass A
        # only pulls the m0/m1 columns; wt pieces are full-n k-slabs (512KB).
        # Pass A (m0+m1, k-interleaved) consumes all of wt spread over its
        # 256 matmuls; passes B/C (m2, m3 solo) run fully resident and
        # stagger the PSUM evictions.
        xt_sb = [[], []]  # per m-half: 8 tiles [P, 4 * (2*P)]
        wt_sb = []  # 32 tiles [P, NT*NFREE]; k-slab k, all 4 n banks
        XW = 2 * P  # xt piece width per k-slab (2 m chunks)
        for kk in range(KT // 4):
            xk = xpool.tile([P, 4 * XW], bf16, tag=f"xa{kk}")
            nc.sync.dma_start(xk[:], xt_ap[0, kk])
            xt_sb[0].append(xk)
            for q in range(4):
                wk = wpool.tile([P, NT * NFREE], bf16, tag=f"w{kk}_{q}")
                nc.sync.dma_start(wk[:], wt_ap[4 * kk + q])
                wt_sb.append(wk)
        for kk in range(KT // 4):
            xk = xpool.tile([P, 4 * XW], bf16, tag=f"xb{kk}")
            nc.sync.dma_start(xk[:], xt_ap[1, kk])
            xt_sb[1].append(xk)

        def lhsT_of(m, k):
            half, mm = divmod(m, 2)
            return xt_sb[half][k // 4][:, (k % 4) * XW + mm * P :][:, :P]

        def emit_evict(m, acc):
            for n in range(NT):
                st = stage.tile([P, NFREE], f32, tag="st", name=f"st{m}_{n}")
                nc.vector.tensor_copy(st[:], acc[:, n * NFREE : (n + 1) * NFREE])
                nc.sync.dma_start(out_ap[m, n], st[:])

        # Pass A: m0 + m1 interleaved per k (spreads wt DMA demand).
        acc01 = [
            psum.tile([P, NT * NFREE], f32, tag="acc", name=f"accA{m}")
            for m in range(2)
        ]
        for k in range(KT):
            wt_tile = wt_sb[k]
            for m in range(2):
                lhsT = lhsT_of(m, k)
                for n in range(NT):
                    nc.tensor.matmul(
                        acc01[m][:, n * NFREE : (n + 1) * NFREE],
                        lhsT,
                        wt_tile[:, n * NFREE : (n + 1) * NFREE],
                        start=(k == 0),
                        stop=(k == KT - 1),
                    )
        for m in range(2):
            emit_evict(m, acc01[m])

        # Passes B/C: m2 then m3, fully resident, staggered eviction.
        for m in range(2, MT):
            acc = psum.tile([P, NT * NFREE], f32, tag="acc", name=f"accB{m}")
            for k in range(KT):
                lhsT = lhsT_of(m, k)
                for n in range(NT):
                    nc.tensor.matmul(
                        acc[:, n * NFREE : (n + 1) * NFREE],
                        lhsT,
                        wt_sb[k][:, n * NFREE : (n + 1) * NFREE],
                        start=(k == 0),
                        stop=(k == KT - 1),
                    )
            emit_evict(m, acc)


def _get_compiled():
    global _compiled_nc
    if _compiled_nc is None:
        import concourse.mybir as mybir
        import concourse.tile as tile
        from concourse import bacc

        nc = bacc.Bacc(
            "TRN2",
            target_bir_lowering=False,
            debug=False,
            num_devices=N_CORES,
        )
        xt = nc.dram_tensor(
            "xt", [2, KT // 4, P, 4 * 2 * P], mybir.dt.bfloat16, kind="ExternalInput"
        )
        wt = nc.dram_tensor(
            "wt", [KT, P, NT * NFREE], mybir.dt.bfloat16, kind="ExternalInput"
        )
        out = nc.dram_tensor(
            "out", [MT, NT, P, NFREE], mybir.dt.float32, kind="ExternalOutput"
        )
        warm = nc.dram_tensor("warm", [P, 8], mybir.dt.float32, kind="ExternalOutput")
        with tile.TileContext(nc) as tc:
            _build_device_kernel(tc, xt.ap(), wt.ap(), out.ap(), warm.ap())
        nc.compile()
        _compiled_nc = nc
    return _compiled_nc


def _prepare_in_maps(inputs):
    x = np.asarray(inputs["x"], np.float32)
    Wt = _build_Wt(
        inputs["spectral_weights"], inputs["engram_values"], inputs["engram_indices"]
    )  # [DIM(k), DIM(d)] f32
    xb = x.astype(ml_dtypes.bfloat16)
    Wtb = Wt.astype(ml_dtypes.bfloat16)
    in_maps = []
    for c in range(N_CORES):
        r, f = divmod(c, COL_GROUPS)
        xcT = np.ascontiguousarray(xb[r * M_C : (r + 1) * M_C, :].T)  # [DIM, M_C]
        # xt[half, kk] [P, 4*256]: col = lane*256 + mm*128 + c maps to
        # xcT[(4kk+lane)*128 + p, (half*2+mm)*128 + c]
        xt = np.ascontiguousarray(
            xcT.reshape(KT // 4, 4, P, 2, 2, P).transpose(3, 0, 2, 1, 4, 5)
        ).reshape(2, KT // 4, P, 4 * 2 * P)
        # wt[k] [P, NT*NFREE] = Wt_c k-slab, all n banks
        wt = np.ascontiguousarray(Wtb[:, f * N_C : (f + 1) * N_C]).reshape(
            KT, P, NT * NFREE
        )
        in_maps.append({"xt": xt, "wt": wt})
    return in_maps


def _run(in_maps, trace=False):
    from concourse.bass_utils import run_bass_kernel_spmd

    nc = _get_compiled()
    return run_bass_kernel_spmd(
        nc, in_maps, core_ids=list(range(N_CORES)), trace=trace
    )


def _assemble(results):
    out = np.empty((BATCH, DIM), np.float32)
    for c in range(N_CORES):
        r, f = divmod(c, COL_GROUPS)
        piece = results[c]["out"]  # [MT, NT, P, NFREE]
        core_out = piece.transpose(0, 2, 1, 3).reshape(M_C, N_C)
        out[r * M_C : (r + 1) * M_C, f * N_C : (f + 1) * N_C] = core_out
    return out


def kernel(**inputs):
    res = _run(_prepare_in_maps(inputs))
    return _assemble(res.results)


# revision 12
# speedup vs baseline: 1.2844x; 1.1001x over previous
"""Trainium2 kernel for nn_AlephPracticalEngine (sparse_attention).

The reference computes out = fhh(fhh(x) * w + gather-einsum) / DIM, which is
linear in x: out = x @ W^T with W = M (diag(w) + S) M / DIM, where M is the
fhh butterfly matrix and S is scatter-added from the engram tables. The
(x-independent) weight fold runs on host; the device does one dense
[2048,4096] x [4096,4096] bf16 matmul, sharded 4 (batch) x 2 (features)
across 8 NeuronCores.

Per core: out_c[512,2048] = x_c[512,4096] @ W_c[2048,4096]^T. Custom tile
kernel: two n-half passes (PSUM = 4 m-tiles x 2 banks per pass) so weight
DMA spreads across the whole run; one stationary load per 2 matmuls;
512KB DMA pieces; HAM-warmup matmuls kept live via a scratch output.
"""

import numpy as np
import ml_dtypes

DIM = 4096
BATCH = 2048
N_CORES = 8
ROW_GROUPS = 4  # batch groups
COL_GROUPS = 2  # output-feature groups
M_C = BATCH // ROW_GROUPS  # 512 batch rows per core
N_C = DIM // COL_GROUPS  # 2048 output features per core

P = 128
NFREE = 512
KT = DIM // P  # 32 k subtiles
MT = M_C // P  # 4 m chunks
NT = N_C // NFREE  # 4 n banks
N_PASS = 2  # n-half passes (2 banks each)
N_WARMUP = 8

_compiled_nc = None


def _fhh_np(x):
    """numpy mirror of the reference fhh butterfly (last axis)."""
    n = x.shape[-1]
    if n == 1:
        return x
    half = n // 2
    left, right = x[..., :half], x[..., half:]
    h_sum = left + right
    h_diff = left - right
    h_diff2 = h_diff + np.roll(h_diff, shift=1, axis=-1)
    return np.concatenate([_fhh_np(h_sum), _fhh_np(h_diff2)], axis=-1)


def _build_Wt(spectral_weights, engram_values, engram_indices, dtype=np.float32):
    """W^T [k, d] for out = x @ W^T, W = M (diag(w) + S) M / DIM."""
    w = np.asarray(spectral_weights, dtype)
    val = np.asarray(engram_values, dtype)
    idx = np.asarray(engram_indices).astype(np.int64)
    D, K = val.shape
    # M^T = fhh(I):  fhh(e_j)[i] = M[i, j]
    M = np.ascontiguousarray(_fhh_np(np.eye(D, dtype=dtype)).T)
    AM = w[:, None] * M
    for k in range(K):
        AM += val[:, k][:, None] * M[idx[:, k], :]
    # W = fhh applied along axis 0 of AM, / D;  W^T = fhh(AM^T) / D
    Wt = _fhh_np(np.ascontiguousarray(AM.T)) / D
    return np.ascontiguousarray(Wt)


def _build_device_kernel(tc, xt_ap, wt_ap, out_ap, warm_ap):
    import concourse.mybir as mybir

    nc = tc.nc
    bf16 = mybir.dt.bfloat16
    f32 = mybir.dt.float32

    with (
        tc.tile_pool(name="const", bufs=1) as const,
        tc.tile_pool(name="xpool", bufs=1) as xpool,
        tc.tile_pool(name="wpool", bufs=1) as wpool,
        tc.tile_pool(name="stage", bufs=6) as stage,
        tc.tile_pool(name="psum", bufs=4, space="PSUM") as psum,
    ):
        # HAM warmup: one live accumulation group with no DMA dependency,
        # kept alive (vs DCE) by a tiny copy-out to a scratch output.
        wz = const.tile([P, NFREE], bf16, tag="wz")
        nc.any.memset(wz[:], 0.0)
        warm_ps = psum.tile([P, NFREE], f32, tag="acc")
        for i in range(N_WARMUP):
            nc.tensor.matmul(
                warm_ps[:],
                wz[:, :P],
                wz[:],
                start=(i == 0),
                stop=(i == N_WARMUP - 1),
            )
        warm_st = stage.tile([P, 8], f32, tag="warm_st")
        nc.vector.tensor_copy(warm_st[:], warm_ps[:, :8])
        nc.sync.dma_start(warm_ap, warm_st[:])

        # Resident loads in consumption order: xt (small, early), then the
        # n-half-A wt pieces, then the n-half-B pieces. Phase 1 streams
        # half A while it loads; phase 2 runs fully resident.
        NH = NT // 2 * NFREE  # 1024 cols = one n-half = 2 PSUM banks
        xt_sb = []  # 8 tiles [P, 4*M_C]; k-slab k at cols (k%4)*M_C
        wt_sb = [[], []]  # per half: 32 tiles [P, NH]
        for kk in range(KT // 4):
            xk = xpool.tile([P, 4 * MT * P], bf16, tag=f"x{kk}")
            nc.sync.dma_start(xk[:], xt_ap[kk])
            xt_sb.append(xk)
        for h in range(2):
            for k in range(KT):
                wk = wpool.tile([P, NH], bf16, tag=f"w{h}_{k}")
                nc.sync.dma_start(wk[:], wt_ap[h, k])
                wt_sb[h].append(wk)

        def lhsT_of(m, k):
            return xt_sb[k // 4][:, (k % 4) * MT * P + m * P :][:, :P]

        def emit_unit_mms(m, h, k, acc):
            lhsT = lhsT_of(m, k)
            for j in range(2):
                nc.tensor.matmul(
                    acc[:, j * NFREE : (j + 1) * NFREE],
                    lhsT,
                    wt_sb[h][k][:, j * NFREE : (j + 1) * NFREE],
                    start=(k == 0),
                    stop=(k == KT - 1),
                )

        def emit_evict(m, h, acc):
            for j in range(2):
                st = stage.tile([P, NFREE], f32, tag="st", name=f"st{m}_{h}_{j}")
                nc.vector.tensor_copy(st[:], acc[:, j * NFREE : (j + 1) * NFREE])
                nc.sync.dma_start(out_ap[m, 2 * h + j], st[:])

        # Phase 1: n-half A, all m interleaved per k. m0 stops 6 matmuls
        # before the phase ends, so its eviction frees a PSUM slot in time
        # for phase 2 to start without a stall.
        accA = [
            psum.tile([P, NH], f32, tag="acc", name=f"accA{m}") for m in range(MT)
        ]
        for k in range(KT):
            for m in range(MT):
                emit_unit_mms(m, 0, k, accA[m])
        for m in range(MT):
            emit_evict(m, 0, accA[m])

        # Phase 2: n-half B, solo-m k-sweeps (fully resident), staggered
        # completions so evictions overlap the next sweep.
        for m in range(MT):
            acc = psum.tile([P, NH], f32, tag="acc", name=f"accB{m}")
            for k in range(KT):
                emit_unit_mms(m, 1, k, acc)
            emit_evict(m, 1, acc)


def _get_compiled():
    global _compiled_nc
    if _compiled_nc is None:
        import concourse.mybir as mybir
        import concourse.tile as tile
        from concourse import bacc

        nc = bacc.Bacc(
            "TRN2",
            target_bir_lowering=False,
            debug=False,
            num_devices=N_CORES,
        )
        xt = nc.dram_tensor(
            "xt", [KT // 4, P, 4 * M_C], mybir.dt.bfloat16, kind="ExternalInput"
        )
        wt = nc.dram_tensor(
            "wt", [2, KT, P, NT // 2 * NFREE], mybir.dt.bfloat16, kind="ExternalInput"
        )
        out = nc.dram_tensor(
            "out", [MT, NT, P, NFREE], mybir.dt.float32, kind="ExternalOutput"
        )
        warm = nc.dram_tensor("warm", [P, 8], mybir.dt.float32, kind="ExternalOutput")
        with tile.TileContext(nc) as tc:
            _build_device_kernel(tc, xt.ap(), wt.ap(), out.ap(), warm.ap())
        nc.compile()
        _compiled_nc = nc
    return _compiled_nc


def _prepare_in_maps(inputs):
    x = np.asarray(inputs["x"], np.float32)
    Wt = _build_Wt(
        inputs["spectral_weights"], inputs["engram_values"], inputs["engram_indices"]
    )  # [DIM(k), DIM(d)] f32
    xb = x.astype(ml_dtypes.bfloat16)
    Wtb = Wt.astype(ml_dtypes.bfloat16)
    in_maps = []
    for c in range(N_CORES):
        r, f = divmod(c, COL_GROUPS)
        xcT = np.ascontiguousarray(xb[r * M_C : (r + 1) * M_C, :].T)  # [DIM, M_C]
        # xt[kk] [P, 4*M_C]: col = lane*M_C + cc maps to xcT[(4kk+lane)*128+p, cc]
        xt = np.ascontiguousarray(
            xcT.reshape(KT // 4, 4, P, M_C).transpose(0, 2, 1, 3)
        ).reshape(KT // 4, P, 4 * M_C)
        # wt[h, k] [P, 1024] = Wt_c k-slab, n-half h
        wc = Wtb[:, f * N_C : (f + 1) * N_C]
        nh = NT // 2 * NFREE
        wt = np.ascontiguousarray(
            wc.reshape(KT, P, 2, nh).transpose(2, 0, 1, 3)
        )
        in_maps.append({"xt": xt, "wt": wt})
    return in_maps


def _run(in_maps, trace=False):
    from concourse.bass_utils import run_bass_kernel_spmd

    nc = _get_compiled()
    return run_bass_kernel_spmd(
        nc, in_maps, core_ids=list(range(N_CORES)), trace=trace
    )


def _assemble(results):
    out = np.empty((BATCH, DIM), np.float32)
    for c in range(N_CORES):
        r, f = divmod(c, COL_GROUPS)
        piece = results[c]["out"]  # [MT, NT, P, NFREE]
        core_out = piece.transpose(0, 2, 1, 3).reshape(M_C, N_C)
        out[r * M_C : (r + 1) * M_C, f * N_C : (f + 1) * N_C] = core_out
    return out


def kernel(**inputs):
    res = _run(_prepare_in_maps(inputs))
    return _assemble(res.results)


# revision 15
# speedup vs baseline: 1.4353x; 1.1176x over previous
"""Trainium2 kernel for nn_AlephPracticalEngine (sparse_attention).

The reference computes out = fhh(fhh(x) * w + gather-einsum) / DIM, which is
linear in x: out = x @ W^T with W = M (diag(w) + S) M / DIM, where M is the
fhh butterfly matrix and S is scatter-added from the engram tables. The
(x-independent) weight fold runs on host; the device does one dense
[2048,4096] x [4096,4096] bf16 matmul, sharded 4 (batch) x 2 (features)
across 8 NeuronCores.

Per core: out_c[512,2048] = x_c[512,4096] @ W_c[2048,4096]^T. Custom tile
kernel: two n-half passes (PSUM = 4 m-tiles x 2 banks per pass) so weight
DMA spreads across the whole run; one stationary load per 2 matmuls;
512KB DMA pieces; HAM-warmup matmuls kept live via a scratch output.
"""

import numpy as np
import ml_dtypes

DIM = 4096
BATCH = 2048
N_CORES = 8
ROW_GROUPS = 4  # batch groups
COL_GROUPS = 2  # output-feature groups
M_C = BATCH // ROW_GROUPS  # 512 batch rows per core
N_C = DIM // COL_GROUPS  # 2048 output features per core

P = 128
NFREE = 512
KT = DIM // P  # 32 k subtiles
MT = M_C // P  # 4 m chunks
NT = N_C // NFREE  # 4 n banks
N_PASS = 2  # n-half passes (2 banks each)
N_WARMUP = 8

_compiled_nc = None


def _fhh_np(x):
    """numpy mirror of the reference fhh butterfly (last axis)."""
    n = x.shape[-1]
    if n == 1:
        return x
    half = n // 2
    left, right = x[..., :half], x[..., half:]
    h_sum = left + right
    h_diff = left - right
    h_diff2 = h_diff + np.roll(h_diff, shift=1, axis=-1)
    return np.concatenate([_fhh_np(h_sum), _fhh_np(h_diff2)], axis=-1)


def _build_Wt(spectral_weights, engram_values, engram_indices, dtype=np.float32):
    """W^T [k, d] for out = x @ W^T, W = M (diag(w) + S) M / DIM."""
    w = np.asarray(spectral_weights, dtype)
    val = np.asarray(engram_values, dtype)
    idx = np.asarray(engram_indices).astype(np.int64)
    D, K = val.shape
    # M^T = fhh(I):  fhh(e_j)[i] = M[i, j]
    M = np.ascontiguousarray(_fhh_np(np.eye(D, dtype=dtype)).T)
    AM = w[:, None] * M
    for k in range(K):
        AM += val[:, k][:, None] * M[idx[:, k], :]
    # W = fhh applied along axis 0 of AM, / D;  W^T = fhh(AM^T) / D
    Wt = _fhh_np(np.ascontiguousarray(AM.T)) / D
    return np.ascontiguousarray(Wt)


def _build_device_kernel(tc, xt_ap, wt_ap, out_ap, warm_ap):
    import concourse.mybir as mybir

    nc = tc.nc
    bf16 = mybir.dt.bfloat16
    f32 = mybir.dt.float32

    with (
        tc.tile_pool(name="const", bufs=1) as const,
        tc.tile_pool(name="xpool", bufs=1) as xpool,
        tc.tile_pool(name="wpool", bufs=1) as wpool,
        tc.tile_pool(name="stage", bufs=6) as stage,
        tc.tile_pool(name="psum", bufs=4, space="PSUM") as psum,
    ):
        # HAM warmup: one live accumulation group with no DMA dependency,
        # kept alive (vs DCE) by a tiny copy-out to a scratch output.
        wz = const.tile([P, NFREE], bf16, tag="wz")
        nc.any.memset(wz[:], 0.0)
        warm_ps = psum.tile([P, NFREE], f32, tag="acc")
        for i in range(N_WARMUP):
            nc.tensor.matmul(
                warm_ps[:],
                wz[:, :P],
                wz[:],
                start=(i == 0),
                stop=(i == N_WARMUP - 1),
            )
        warm_st = stage.tile([P, 8], f32, tag="warm_st")
        nc.vector.tensor_copy(warm_st[:], warm_ps[:, :8])
        nc.sync.dma_start(warm_ap, warm_st[:])

        # Resident loads in consumption order: xt (small, early), then the
        # n-half-A wt pieces, then the n-half-B pieces. Phase 1 streams
        # half A while it loads; phase 2 runs fully resident.
        NH = NT // 2 * NFREE  # 1024 cols = one n-half = 2 PSUM banks
        xt_sb = []  # 16 tiles [P, 2*M_C] (2KB pitch); k-slab k at (k%2)*M_C
        wt_sb = [[], []]  # per half: 32 tiles [P, NH]
        # Interleave xt with the half-A wt pieces in consumption order so
        # the matmul stream never outruns the loads.
        for kk in range(KT // 2):
            xk = xpool.tile([P, 2 * MT * P], bf16, tag=f"x{kk}")
            nc.sync.dma_start(xk[:], xt_ap[kk])
            xt_sb.append(xk)
            for q in range(2):
                k = 2 * kk + q
                wk = wpool.tile([P, NH], bf16, tag=f"w0_{k}")
                nc.sync.dma_start(wk[:], wt_ap[0, k])
                wt_sb[0].append(wk)
        for k in range(KT):
            wk = wpool.tile([P, NH], bf16, tag=f"w1_{k}")
            nc.sync.dma_start(wk[:], wt_ap[1, k])
            wt_sb[1].append(wk)

        def lhsT_of(m, k):
            return xt_sb[k // 2][:, (k % 2) * MT * P + m * P :][:, :P]

        def emit_unit_mms(m, h, k, acc):
            lhsT = lhsT_of(m, k)
            for j in range(2):
                nc.tensor.matmul(
                    acc[:, j * NFREE : (j + 1) * NFREE],
                    lhsT,
                    wt_sb[h][k][:, j * NFREE : (j + 1) * NFREE],
                    start=(k == 0),
                    stop=(k == KT - 1),
                )

        def emit_evict(m, h, acc):
            for j in range(2):
                st = stage.tile([P, NFREE], f32, tag="st", name=f"st{m}_{h}_{j}")
                nc.vector.tensor_copy(st[:], acc[:, j * NFREE : (j + 1) * NFREE])
                nc.sync.dma_start(out_ap[m, 2 * h + j], st[:])

        # Phase 1: n-half A, all m interleaved per k. m0 stops 6 matmuls
        # before the phase ends, so its eviction frees a PSUM slot in time
        # for phase 2 to start without a stall.
        accA = [
            psum.tile([P, NH], f32, tag="acc", name=f"accA{m}") for m in range(MT)
        ]
        for k in range(KT):
            for m in range(MT):
                emit_unit_mms(m, 0, k, accA[m])
        for m in range(MT):
            emit_evict(m, 0, accA[m])

        # Phase 2: n-half B, solo-m k-sweeps (fully resident), staggered
        # completions so evictions overlap the next sweep.
        for m in range(MT):
            acc = psum.tile([P, NH], f32, tag="acc", name=f"accB{m}")
            for k in range(KT):
                emit_unit_mms(m, 1, k, acc)
            emit_evict(m, 1, acc)


def _get_compiled():
    global _compiled_nc
    if _compiled_nc is None:
        import concourse.mybir as mybir
        import concourse.tile as tile
        from concourse import bacc

        nc = bacc.Bacc(
            "TRN2",
            target_bir_lowering=False,
            debug=False,
            num_devices=N_CORES,
        )
        xt = nc.dram_tensor(
            "xt", [KT // 2, P, 2 * M_C], mybir.dt.bfloat16, kind="ExternalInput"
        )
        wt = nc.dram_tensor(
            "wt", [2, KT, P, NT // 2 * NFREE], mybir.dt.bfloat16, kind="ExternalInput"
        )
        out = nc.dram_tensor(
            "out", [MT, NT, P, NFREE], mybir.dt.float32, kind="ExternalOutput"
        )
        warm = nc.dram_tensor("warm", [P, 8], mybir.dt.float32, kind="ExternalOutput")
        with tile.TileContext(nc) as tc:
            _build_device_kernel(tc, xt.ap(), wt.ap(), out.ap(), warm.ap())
        nc.compile()
        _compiled_nc = nc
    return _compiled_nc


def _prepare_in_maps(inputs):
    x = np.asarray(inputs["x"], np.float32)
    Wt = _build_Wt(
        inputs["spectral_weights"], inputs["engram_values"], inputs["engram_indices"]
    )  # [DIM(k), DIM(d)] f32
    xb = x.astype(ml_dtypes.bfloat16)
    Wtb = Wt.astype(ml_dtypes.bfloat16)
    in_maps = []
    for c in range(N_CORES):
        r, f = divmod(c, COL_GROUPS)
        xcT = np.ascontiguousarray(xb[r * M_C : (r + 1) * M_C, :].T)  # [DIM, M_C]
        # xt[kk] [P, 2*M_C]: col = lane*M_C + cc maps to xcT[(2kk+lane)*128+p, cc]
        xt = np.ascontiguousarray(
            xcT.reshape(KT // 2, 2, P, M_C).transpose(0, 2, 1, 3)
        ).reshape(KT // 2, P, 2 * M_C)
        # wt[h, k] [P, 1024] = Wt_c k-slab, n-half h
        wc = Wtb[:, f * N_C : (f + 1) * N_C]
        nh = NT // 2 * NFREE
        wt = np.ascontiguousarray(
            wc.reshape(KT, P, 2, nh).transpose(2, 0, 1, 3)
        )
        in_maps.append({"xt": xt, "wt": wt})
    return in_maps


def _run(in_maps, trace=False):
    from concourse.bass_utils import run_bass_kernel_spmd

    nc = _get_compiled()
    return run_bass_kernel_spmd(
        nc, in_maps, core_ids=list(range(N_CORES)), trace=trace
    )


def _assemble(results):
    out = np.empty((BATCH, DIM), np.float32)
    for c in range(N_CORES):
        r, f = divmod(c, COL_GROUPS)
        piece = results[c]["out"]  # [MT, NT, P, NFREE]
        core_out = piece.transpose(0, 2, 1, 3).reshape(M_C, N_C)
        out[r * M_C : (r + 1) * M_C, f * N_C : (f + 1) * N_C] = core_out
    return out


def kernel(**inputs):
    res = _run(_prepare_in_maps(inputs))
    return _assemble(res.results)
